# revision 15
# baseline (speedup 1.0000x reference)
"""Distributed embedding-lookup kernel (doc2vec PV-DM forward) for 8 trn2 cores.

Math (per batch element b):
    inputs[b,:]  = paragraph_matrix[doc_ids[b]] + mean_c word_matrix[context_ids[b,c]]
    result[b,s]  = dot(inputs[b,:], outputs[:, sample_ids[b,s]])

Sharding: data-parallel over batch (2048 rows/core); paragraph/word/output
tables replicated to every core.  `outputs` is transposed host-side to
[N_WORDS, D] so sampled columns become contiguous 512B row gathers.

Performance note (all verified on this hardware):
  * An indirect DMA consumes exactly ONE index per partition per instruction.
    A [128,K] offset AP does NOT gather K rows -- it streams K*D contiguous
    elements from the first index.  The batched dma_gather firmware (custom
    GPSIMD ucode) is absent from this bedrock image and crashes the device.
  * Hence the gather needs 19 SWDGE ops per 128-row tile, 304 per core, and
    each op costs ~1.1us of serial Pool-engine descriptor-generation time
    (994ns fixed + 0.34ns/descriptor).  That is a ~335us floor that the DVE
    compute and stores fully hide under.
  * Gathers MUST land in a whole tile of their own: slice-writes into a
    shared pooled tile serialize on DMA-completion semaphores and cost
    ~1.66us/op instead of ~1.1us/op (measured: 503us vs 335us per rep).
"""

import sys

if "/opt/trn_rl_repo" not in sys.path:
    sys.path.insert(0, "/opt/trn_rl_repo")

import numpy as np

N_CORES = 8
B, C, S = 16384, 8, 10
D = 128
P = 128
N_DOCS, N_WORDS = 200000, 100000
BS = B // N_CORES  # 2048 batch rows per core
T = BS // P        # 16 tiles of 128 rows per core

_COMPILED = {}
LAST_RESULT = None  # BassKernelResults of the most recent run (for test harness)


def _build_program(reps: int = 1, num_devices: int = N_CORES):
    import concourse.bass as bass
    import concourse.tile as tile
    from concourse import bacc, mybir
    from contextlib import ExitStack

    f32 = mybir.dt.float32
    i32 = mybir.dt.int32

    nc = bacc.Bacc(
        "TRN2",
        target_bir_lowering=False,
        debug=False,
        enable_asserts=False,
        num_devices=num_devices,
    )

    par_d = nc.dram_tensor("par", [N_DOCS, D], f32, kind="ExternalInput").ap()
    wrd_d = nc.dram_tensor("wrd", [N_WORDS, D], f32, kind="ExternalInput").ap()
    outT_d = nc.dram_tensor("outT", [N_WORDS, D], f32, kind="ExternalInput").ap()
    idx_doc_d = nc.dram_tensor("idx_doc", [P, T], i32, kind="ExternalInput").ap()
    idx_ctx_d = nc.dram_tensor("idx_ctx", [P, T * C], i32, kind="ExternalInput").ap()
    idx_smp_d = nc.dram_tensor("idx_smp", [P, T * S], i32, kind="ExternalInput").ap()
    res_d = nc.dram_tensor("res", [BS, S], f32, kind="ExternalOutput").ap()

    with tile.TileContext(nc) as tc, ExitStack() as ctx:
        idxp = ctx.enter_context(tc.tile_pool(name="idxp", bufs=1))
        gat = ctx.enter_context(tc.tile_pool(name="gat", bufs=4))
        cmp_p = ctx.enter_context(tc.tile_pool(name="cmp", bufs=4))
        outp = ctx.enter_context(tc.tile_pool(name="outp", bufs=3))

        idx_doc = idxp.tile([P, T], i32)
        nc.sync.dma_start(out=idx_doc[:], in_=idx_doc_d)
        idx_ctx = idxp.tile([P, T * C], i32)
        nc.sync.dma_start(out=idx_ctx[:], in_=idx_ctx_d)
        idx_smp = idxp.tile([P, T * S], i32)
        nc.sync.dma_start(out=idx_smp[:], in_=idx_smp_d)

        for _rep in range(reps):
            for t in range(T):
                # HW indirect DMA consumes one index per partition per
                # instruction, and slice-writes into a shared pooled tile
                # serialize badly — so every gathered row-block gets a whole
                # tile of its own.
                par = gat.tile([P, D], f32, tag="par")
                nc.gpsimd.indirect_dma_start(
                    out=par[:],
                    out_offset=None,
                    in_=par_d,
                    in_offset=bass.IndirectOffsetOnAxis(
                        ap=idx_doc[:, t : t + 1], axis=0
                    ),
                )
                ctxts = []
                for c in range(C):
                    ct = gat.tile([P, D], f32, tag=f"ctx{c}")
                    nc.gpsimd.indirect_dma_start(
                        out=ct[:],
                        out_offset=None,
                        in_=wrd_d,
                        in_offset=bass.IndirectOffsetOnAxis(
                            ap=idx_ctx[:, t * C + c : t * C + c + 1], axis=0
                        ),
                    )
                    ctxts.append(ct)
                smpts = []
                for s in range(S):
                    st = gat.tile([P, D], f32, tag=f"smp{s}")
                    nc.gpsimd.indirect_dma_start(
                        out=st[:],
                        out_offset=None,
                        in_=outT_d,
                        in_offset=bass.IndirectOffsetOnAxis(
                            ap=idx_smp[:, t * S + s : t * S + s + 1], axis=0
                        ),
                    )
                    smpts.append(st)

                # acc = sum_c ctx ; inp = acc/C + par  (tree-sum: 8->4->2->1)
                t1 = []
                for i in range(4):
                    a = cmp_p.tile([P, D], f32, tag=f"a{i}")
                    nc.vector.tensor_add(
                        out=a[:], in0=ctxts[2 * i][:], in1=ctxts[2 * i + 1][:]
                    )
                    t1.append(a)
                b0 = cmp_p.tile([P, D], f32, tag="b0")
                nc.vector.tensor_add(out=b0[:], in0=t1[0][:], in1=t1[1][:])
                b1 = cmp_p.tile([P, D], f32, tag="b1")
                nc.vector.tensor_add(out=b1[:], in0=t1[2][:], in1=t1[3][:])
                acc = cmp_p.tile([P, D], f32, tag="acc")
                nc.vector.tensor_add(out=acc[:], in0=b0[:], in1=b1[:])
                inp = cmp_p.tile([P, D], f32, tag="inp")
                nc.vector.scalar_tensor_tensor(
                    out=inp[:],
                    in0=acc[:],
                    scalar=1.0 / C,
                    in1=par[:],
                    op0=mybir.AluOpType.mult,
                    op1=mybir.AluOpType.add,
                )
                # red[p,s] = sum_d smp_s[p,d] * inp[p,d]
                prod = cmp_p.tile([P, S * D], f32, tag="prod")
                red = outp.tile([P, S], f32, tag="red")
                for s in range(S):
                    nc.vector.scalar_tensor_tensor(
                        out=prod[:, s * D : (s + 1) * D],
                        in0=smpts[s][:],
                        scalar=1.0,
                        in1=inp[:],
                        op0=mybir.AluOpType.mult,
                        op1=mybir.AluOpType.mult,
                        accum_out=red[:, s : s + 1],
                    )
                nc.sync.dma_start(out=res_d[t * P : (t + 1) * P, :], in_=red[:])

    nc.compile()
    return nc


def _get_program():
    if "nc" not in _COMPILED:
        _COMPILED["nc"] = _build_program()
    return _COMPILED["nc"]


def _tile_major(idx: np.ndarray) -> np.ndarray:
    """[BS, k] int -> [P, T*k] int32 where out[p, t*k+c] = idx[t*P+p, c]."""
    idx = np.asarray(idx)
    if idx.ndim == 1:
        idx = idx[:, None]
    k = idx.shape[1]
    return np.ascontiguousarray(
        idx.reshape(T, P, k).transpose(1, 0, 2).reshape(P, T * k).astype(np.int32)
    )


def kernel(
    doc_ids,
    context_ids,
    sample_ids,
    paragraph_matrix,
    word_matrix,
    outputs,
) -> np.ndarray:
    global LAST_RESULT
    from concourse.bass_utils import run_bass_kernel_spmd

    nc = _get_program()

    par = np.ascontiguousarray(np.asarray(paragraph_matrix, dtype=np.float32))
    wrd = np.ascontiguousarray(np.asarray(word_matrix, dtype=np.float32))
    outT = np.ascontiguousarray(np.asarray(outputs, dtype=np.float32).T)
    doc_ids = np.asarray(doc_ids)
    context_ids = np.asarray(context_ids)
    sample_ids = np.asarray(sample_ids)

    in_maps = []
    for k in range(N_CORES):
        sl = slice(k * BS, (k + 1) * BS)
        in_maps.append(
            {
                "par": par,
                "wrd": wrd,
                "outT": outT,
                "idx_doc": _tile_major(doc_ids[sl]),
                "idx_ctx": _tile_major(context_ids[sl]),
                "idx_smp": _tile_major(sample_ids[sl]),
            }
        )

    LAST_RESULT = run_bass_kernel_spmd(nc, in_maps, list(range(N_CORES)))
    out = np.concatenate(
        [LAST_RESULT.results[k]["res"] for k in range(N_CORES)], axis=0
    )
    return out.astype(np.float32)


# revision 16
# speedup vs baseline: 1.0016x; 1.0016x over previous
"""Distributed embedding-lookup kernel (doc2vec PV-DM forward) for 8 trn2 cores.

Math (per batch element b):
    inputs[b,:]  = paragraph_matrix[doc_ids[b]] + mean_c word_matrix[context_ids[b,c]]
    result[b,s]  = dot(inputs[b,:], outputs[:, sample_ids[b,s]])

Sharding: data-parallel over batch (2048 rows/core); paragraph/word/output
tables replicated to every core.  `outputs` is transposed host-side to
[N_WORDS, D] so sampled columns become contiguous 512B row gathers.

Performance note (all verified on this hardware):
  * An indirect DMA consumes exactly ONE index per partition per instruction.
    A [128,K] offset AP does NOT gather K rows -- it streams K*D contiguous
    elements from the first index.  The batched dma_gather firmware (custom
    GPSIMD ucode) is absent from this bedrock image and crashes the device.
  * Hence the gather needs 19 SWDGE ops per 128-row tile, 304 per core, and
    each op costs ~1.1us of serial Pool-engine descriptor-generation time
    (994ns fixed + 0.34ns/descriptor).  That is a ~335us floor that the DVE
    compute and stores fully hide under.
  * Gathers MUST land in a whole tile of their own: slice-writes into a
    shared pooled tile serialize on DMA-completion semaphores and cost
    ~1.66us/op instead of ~1.1us/op (measured: 503us vs 335us per rep).
"""

import sys

if "/opt/trn_rl_repo" not in sys.path:
    sys.path.insert(0, "/opt/trn_rl_repo")

import numpy as np

N_CORES = 8
B, C, S = 16384, 8, 10
D = 128
P = 128
N_DOCS, N_WORDS = 200000, 100000
BS = B // N_CORES  # 2048 batch rows per core
T = BS // P        # 16 tiles of 128 rows per core

_COMPILED = {}
LAST_RESULT = None  # BassKernelResults of the most recent run (for test harness)


def _build_program(reps: int = 1, num_devices: int = N_CORES):
    import concourse.bass as bass
    import concourse.tile as tile
    from concourse import bacc, mybir
    from contextlib import ExitStack

    f32 = mybir.dt.float32
    i32 = mybir.dt.int32

    nc = bacc.Bacc(
        "TRN2",
        target_bir_lowering=False,
        debug=False,
        enable_asserts=False,
        num_devices=num_devices,
    )

    par_d = nc.dram_tensor("par", [N_DOCS, D], f32, kind="ExternalInput").ap()
    wrd_d = nc.dram_tensor("wrd", [N_WORDS, D], f32, kind="ExternalInput").ap()
    outT_d = nc.dram_tensor("outT", [N_WORDS, D], f32, kind="ExternalInput").ap()
    idx_doc_d = nc.dram_tensor("idx_doc", [P, T], i32, kind="ExternalInput").ap()
    idx_ctx_d = nc.dram_tensor("idx_ctx", [P, T * C], i32, kind="ExternalInput").ap()
    idx_smp_d = nc.dram_tensor("idx_smp", [P, T * S], i32, kind="ExternalInput").ap()
    res_d = nc.dram_tensor("res", [BS, S], f32, kind="ExternalOutput").ap()

    with tile.TileContext(nc) as tc, ExitStack() as ctx:
        idxp = ctx.enter_context(tc.tile_pool(name="idxp", bufs=1))
        gat = ctx.enter_context(tc.tile_pool(name="gat", bufs=4))
        cmp_p = ctx.enter_context(tc.tile_pool(name="cmp", bufs=4))
        outp = ctx.enter_context(tc.tile_pool(name="outp", bufs=3))

        idx_doc = idxp.tile([P, T], i32)
        nc.sync.dma_start(out=idx_doc[:], in_=idx_doc_d)
        idx_ctx = idxp.tile([P, T * C], i32)
        nc.sync.dma_start(out=idx_ctx[:], in_=idx_ctx_d)
        idx_smp = idxp.tile([P, T * S], i32)
        nc.sync.dma_start(out=idx_smp[:], in_=idx_smp_d)

        for _rep in range(reps):
            for t in range(T):
                # HW indirect DMA consumes one index per partition per
                # instruction, and slice-writes into a shared pooled tile
                # serialize badly — so every gathered row-block gets a whole
                # tile of its own.
                par = gat.tile([P, D], f32, tag="par")
                nc.gpsimd.indirect_dma_start(
                    out=par[:],
                    out_offset=None,
                    in_=par_d,
                    in_offset=bass.IndirectOffsetOnAxis(
                        ap=idx_doc[:, t : t + 1], axis=0
                    ),
                )
                ctxts = []
                for c in range(C):
                    ct = gat.tile([P, D], f32, tag=f"ctx{c}")
                    nc.gpsimd.indirect_dma_start(
                        out=ct[:],
                        out_offset=None,
                        in_=wrd_d,
                        in_offset=bass.IndirectOffsetOnAxis(
                            ap=idx_ctx[:, t * C + c : t * C + c + 1], axis=0
                        ),
                    )
                    ctxts.append(ct)
                smpts = []
                for s in range(S):
                    st = gat.tile([P, D], f32, tag=f"smp{s}")
                    nc.gpsimd.indirect_dma_start(
                        out=st[:],
                        out_offset=None,
                        in_=outT_d,
                        in_offset=bass.IndirectOffsetOnAxis(
                            ap=idx_smp[:, t * S + s : t * S + s + 1], axis=0
                        ),
                    )
                    smpts.append(st)

                # acc = sum_c ctx ; inp = acc/C + par
                acc = cmp_p.tile([P, D], f32, tag="acc")
                nc.vector.tensor_add(out=acc[:], in0=ctxts[0][:], in1=ctxts[1][:])
                for c in range(2, C):
                    nc.vector.tensor_add(out=acc[:], in0=acc[:], in1=ctxts[c][:])
                inp = cmp_p.tile([P, D], f32, tag="inp")
                nc.vector.scalar_tensor_tensor(
                    out=inp[:],
                    in0=acc[:],
                    scalar=1.0 / C,
                    in1=par[:],
                    op0=mybir.AluOpType.mult,
                    op1=mybir.AluOpType.add,
                )
                # red[p,s] = sum_d smp_s[p,d] * inp[p,d]
                prod = cmp_p.tile([P, S * D], f32, tag="prod")
                red = outp.tile([P, S], f32, tag="red")
                for s in range(S):
                    nc.vector.scalar_tensor_tensor(
                        out=prod[:, s * D : (s + 1) * D],
                        in0=smpts[s][:],
                        scalar=1.0,
                        in1=inp[:],
                        op0=mybir.AluOpType.mult,
                        op1=mybir.AluOpType.mult,
                        accum_out=red[:, s : s + 1],
                    )
                nc.sync.dma_start(out=res_d[t * P : (t + 1) * P, :], in_=red[:])

    nc.compile()
    return nc


def _get_program():
    if "nc" not in _COMPILED:
        _COMPILED["nc"] = _build_program()
    return _COMPILED["nc"]


def _tile_major(idx: np.ndarray) -> np.ndarray:
    """[BS, k] int -> [P, T*k] int32 where out[p, t*k+c] = idx[t*P+p, c]."""
    idx = np.asarray(idx)
    if idx.ndim == 1:
        idx = idx[:, None]
    k = idx.shape[1]
    return np.ascontiguousarray(
        idx.reshape(T, P, k).transpose(1, 0, 2).reshape(P, T * k).astype(np.int32)
    )


def kernel(
    doc_ids,
    context_ids,
    sample_ids,
    paragraph_matrix,
    word_matrix,
    outputs,
) -> np.ndarray:
    global LAST_RESULT
    from concourse.bass_utils import run_bass_kernel_spmd

    nc = _get_program()

    par = np.ascontiguousarray(np.asarray(paragraph_matrix, dtype=np.float32))
    wrd = np.ascontiguousarray(np.asarray(word_matrix, dtype=np.float32))
    outT = np.ascontiguousarray(np.asarray(outputs, dtype=np.float32).T)
    doc_ids = np.asarray(doc_ids)
    context_ids = np.asarray(context_ids)
    sample_ids = np.asarray(sample_ids)

    in_maps = []
    for k in range(N_CORES):
        sl = slice(k * BS, (k + 1) * BS)
        in_maps.append(
            {
                "par": par,
                "wrd": wrd,
                "outT": outT,
                "idx_doc": _tile_major(doc_ids[sl]),
                "idx_ctx": _tile_major(context_ids[sl]),
                "idx_smp": _tile_major(sample_ids[sl]),
            }
        )

    LAST_RESULT = run_bass_kernel_spmd(nc, in_maps, list(range(N_CORES)))
    out = np.concatenate(
        [LAST_RESULT.results[k]["res"] for k in range(N_CORES)], axis=0
    )
    return out.astype(np.float32)
